# revision 4
# baseline (speedup 1.0000x reference)
"""RetinaFace-style multi-task loss on 8 Trainium NeuronCores (data-parallel).

Two-phase device pipeline to avoid shipping the 1.25 GB ldm_regressions tensor
through the interconnect when only ~200 positive-anchor rows per sample
contribute to the bbox/landmark losses:

  Phase A (device, pmap over 8 cores x 2 samples): full A x 32 IoU,
    pos/neg classification, hard-negative-mined classification loss
    (top-k sum via 16-way threshold search -- no sort). Exports a single
    uint8 plane per anchor: matched-GT index (low bits) | pos flag (bit 7).
  Host: compact positive indices, slice the needed rows of
    bbox_regressions / ldm_regressions / anchors.
  Phase B (device, pmap): gathers GT boxes/landmarks from annotations via
    one-hot matmul, SmoothL1 bbox loss + wing landmark loss on [256]-row tiles.

All math fp32, mirroring the reference formulas.
"""
import numpy as np

_B, _A, _N = 16, 102400, 32
_NC = 8
_K = 256  # max positives per sample (observed ~200; asserted at runtime)
_OMEGA, _EPS = 3.0, 2.0
_WING_C = _OMEGA - _OMEGA * float(np.log(1.0 + _OMEGA / _EPS))

_fns = None


def _build():
    global _fns
    if _fns is not None:
        return _fns
    import jax
    import jax.numpy as jnp

    # ---------------- phase A ----------------
    def phase_a(cls1, ann, anchor):
        # cls1 [A] (=classifications[:,1]), ann [32,200], anchor [A,4]
        aw = anchor[:, 2] - anchor[:, 0]
        ah = anchor[:, 3] - anchor[:, 1]
        valid = ann[:, 0] > 0
        boxes = ann[:, :4]
        has_gt = jnp.any(valid)

        barea = (boxes[:, 2] - boxes[:, 0]) * (boxes[:, 3] - boxes[:, 1])
        iw = jnp.minimum(anchor[:, 2][:, None], boxes[None, :, 2]) - jnp.maximum(
            anchor[:, 0][:, None], boxes[None, :, 0])
        ih = jnp.minimum(anchor[:, 3][:, None], boxes[None, :, 3]) - jnp.maximum(
            anchor[:, 1][:, None], boxes[None, :, 1])
        iw = jnp.clip(iw, 0.0)
        ih = jnp.clip(ih, 0.0)
        ua = jnp.clip((aw * ah)[:, None] + barea[None, :] - iw * ih, 1e-8)
        iou = iw * ih / ua
        iou = jnp.where(valid[None, :], iou, -1.0)
        iou_max = iou.max(axis=1)

        j32 = jnp.arange(32, dtype=jnp.int32)
        idxs = jnp.where(iou == iou_max[:, None], j32[None, :], 99)
        iou_arg = idxs.min(axis=1)

        neg = iou_max < 0.4
        pos = iou_max >= 0.7
        packed = (iou_arg.astype(jnp.uint8)
                  | (pos.astype(jnp.uint8) << 7))
        npos = pos.sum()
        nneg = neg.sum()
        keep = jnp.minimum(nneg, 3 * npos)

        # hard-negative mining: sum of top-`keep` scores via 16-way search
        v = jnp.where(neg, -cls1, jnp.float32(-1e2))
        ks = jnp.arange(16, dtype=jnp.float32)

        def body(_, s):
            lo, hi = s
            t = lo + (ks + 1.0) * ((hi - lo) / 17.0)
            c = (v[:, None] >= t[None, :]).sum(axis=0)
            big = c >= keep
            lo2 = jnp.max(jnp.where(big, t, lo))
            hi2 = jnp.min(jnp.where(big, hi, t))
            return lo2, hi2

        lo, _hi = jax.lax.fori_loop(
            0, 5, body, (jnp.float32(-1e2), jnp.float32(64.0)))
        c_lo = ((v >= lo).sum()).astype(jnp.float32)
        s_lo = jnp.where(v >= lo, v, 0.0).sum()
        keep_f = keep.astype(jnp.float32)
        neg_mean = (s_lo - (c_lo - keep_f) * lo) / jnp.maximum(keep_f, 1.0)
        return neg_mean, packed

    def phase_a_core(cls1, ann, anchor):
        return jax.vmap(phase_a, in_axes=(0, 0, None))(cls1, ann, anchor)

    # ---------------- phase B ----------------
    def phase_b(breg, lreg, anc, ann, gt, rowv, npos, has_gt, cls0, neg_mean):
        # breg [K,4], lreg [K,196], anc [K,4], ann [32,200], gt [K] int32
        onehot = (jnp.arange(32, dtype=jnp.int32)[None, :]
                  == gt[:, None]).astype(jnp.float32)
        gb = jnp.einsum('kj,jc->kc', onehot, ann[:, :4],
                        preferred_element_type=jnp.float32)
        gl = jnp.einsum('kj,jc->kc', onehot, ann[:, 4:],
                        preferred_element_type=jnp.float32)

        aw = anc[:, 2] - anc[:, 0]
        ah = anc[:, 3] - anc[:, 1]
        acx = anc[:, 0] + 0.5 * aw
        acy = anc[:, 1] + 0.5 * ah
        gw = gb[:, 2] - gb[:, 0]
        gh = gb[:, 3] - gb[:, 1]
        gcx = gb[:, 0] + 0.5 * gw
        gcy = gb[:, 1] + 0.5 * gh
        tdx = (gcx - acx) / (aw + 1e-14)
        tdy = (gcy - acy) / (ah + 1e-14)
        tdw = jnp.log(jnp.where(gw > 0, gw / aw, 1.0))
        tdh = jnp.log(jnp.where(gh > 0, gh / ah, 1.0))
        bbox_scale = jnp.array([0.1, 0.1, 0.2, 0.2], jnp.float32)
        btgt = jnp.stack([tdx, tdy, tdw, tdh], axis=1) / bbox_scale
        d = jnp.abs(btgt - breg)
        sl1 = jnp.where(d < 1.0, 0.5 * d * d, d - 0.5)
        npos_f = jnp.maximum(npos, 1.0)
        bbox_loss = jnp.where(
            (has_gt > 0) & (npos > 0),
            jnp.where(rowv[:, None] > 0, sl1, 0.0).sum() / (npos_f * 4.0), 0.0)

        even = (jnp.arange(196) % 2) == 0
        ctr = jnp.where(even, acx[:, None], acy[:, None])
        den = jnp.where(even, aw[:, None], ah[:, None]) + 1e-14
        s = jnp.concatenate(
            [jnp.ones(68, jnp.float32), 3.0 * jnp.ones(128, jnp.float32)])
        lposv = (rowv > 0) & (gl.sum(axis=1) > 0)
        nl = lposv.sum()
        ltgt = (gl - ctr) / den / 0.1
        dd = jnp.abs(ltgt * s - lreg * s)
        wing = jnp.where(dd < _OMEGA, _OMEGA * jnp.log1p(dd / _EPS), dd - _WING_C)
        ldm_loss = jnp.where(
            (has_gt > 0) & (nl > 0),
            jnp.where(lposv[:, None], wing, 0.0).sum() /
            (jnp.maximum(nl, 1) * 196), 0.0)
        pos_mean = jnp.where(rowv > 0, -cls0, 0.0).sum() / npos_f
        cls_loss = jnp.where((has_gt > 0) & (npos > 0),
                             pos_mean + neg_mean, 0.0)
        return cls_loss, bbox_loss, ldm_loss

    def phase_b_core(*a):
        return jax.vmap(phase_b)(*a)

    _fns = (jax.pmap(phase_a_core, in_axes=(0, 0, 0)),
            jax.pmap(phase_b_core))
    return _fns


def kernel(classifications, bbox_regressions, ldm_regressions, anchors,
           annotations):
    fa, fb = _build()
    spb = _B // _NC
    cls_h = np.asarray(classifications, np.float32)
    cls1 = np.ascontiguousarray(cls_h[:, :, 1]).reshape(_NC, spb, _A)
    ann_h = np.asarray(annotations, np.float32)
    ann = ann_h.reshape(_NC, spb, _N, 200)
    anc_full = np.asarray(anchors, np.float32)[0]
    anc8 = np.broadcast_to(anc_full, (_NC, _A, 4))

    neg_mean, packed = fa(cls1, ann, anc8)
    neg_mean = np.asarray(neg_mean, np.float32).reshape(_B)
    packed = np.asarray(packed).reshape(_B, _A)
    pos_m = (packed >> 7) & 1
    arg_m = packed & 0x3F

    breg_h = np.asarray(bbox_regressions, np.float32)
    lreg_h = np.asarray(ldm_regressions, np.float32)

    breg_g = np.zeros((_B, _K, 4), np.float32)
    lreg_g = np.zeros((_B, _K, 196), np.float32)
    anc_g = np.zeros((_B, _K, 4), np.float32)
    anc_g[:, :, 2:] = 1.0  # pad anchors keep logs/denominators finite
    gt_g = np.full((_B, _K), 99, np.int32)  # 99 -> all-zero one-hot row
    rowv = np.zeros((_B, _K), np.float32)
    cls0_g = np.zeros((_B, _K), np.float32)
    nposs = np.zeros((_B,), np.float32)
    hasgt = np.zeros((_B,), np.float32)

    for b in range(_B):
        idx = np.nonzero(pos_m[b])[0]
        n = idx.size
        assert n <= _K, f'npos={n} exceeds K={_K}'
        breg_g[b, :n] = breg_h[b, idx]
        lreg_g[b, :n] = lreg_h[b, idx]
        anc_g[b, :n] = anc_full[idx]
        gt_g[b, :n] = arg_m[b, idx]
        cls0_g[b, :n] = cls_h[b, idx, 0]
        rowv[b, :n] = 1.0
        nposs[b] = n
        hasgt[b] = float((ann_h[b, :, 0] > 0).any())

    sh = lambda x: x.reshape((_NC, spb) + x.shape[1:])
    cls_loss, bbox_loss, ldm_loss = fb(
        sh(breg_g), sh(lreg_g), sh(anc_g), sh(ann_h), sh(gt_g), sh(rowv),
        sh(nposs), sh(hasgt), sh(cls0_g), sh(neg_mean))
    return (np.asarray(cls_loss, np.float32).reshape(_B),
            np.asarray(bbox_loss, np.float32).reshape(_B),
            np.asarray(ldm_loss, np.float32).reshape(_B))


# revision 5
# speedup vs baseline: 1.0861x; 1.0861x over previous
"""RetinaFace-style multi-task loss on 8 Trainium NeuronCores (data-parallel).

Two-phase device pipeline to avoid shipping the 1.25 GB ldm_regressions tensor
through the interconnect when only ~200 positive-anchor rows per sample
contribute to the bbox/landmark losses:

  Phase A (device, pmap over 8 cores x 2 samples): full A x 32 IoU,
    pos/neg classification, hard-negative-mined classification loss
    (top-k sum via 16-way threshold search -- no sort). Exports a single
    uint8 plane per anchor: matched-GT index (low bits) | pos flag (bit 7).
  Host: compact positive indices, slice the needed rows of
    bbox_regressions / ldm_regressions / anchors.
  Phase B (device, pmap): gathers GT boxes/landmarks from annotations via
    one-hot matmul, SmoothL1 bbox loss + wing landmark loss on [256]-row tiles.

All math fp32, mirroring the reference formulas.
"""
import numpy as np

_B, _A, _N = 16, 102400, 32
_NC = 8
_K = 256  # max positives per sample (observed ~200; asserted at runtime)
_OMEGA, _EPS = 3.0, 2.0
_WING_C = _OMEGA - _OMEGA * float(np.log(1.0 + _OMEGA / _EPS))

_fns = None


def _build():
    global _fns
    if _fns is not None:
        return _fns
    import jax
    import jax.numpy as jnp

    # ---------------- phase A ----------------
    def phase_a(cls1, ann, anchor):
        # cls1 [A] (=classifications[:,1]), ann [32,200], anchor [A,4]
        aw = anchor[:, 2] - anchor[:, 0]
        ah = anchor[:, 3] - anchor[:, 1]
        valid = ann[:, 0] > 0
        boxes = ann[:, :4]
        has_gt = jnp.any(valid)

        barea = (boxes[:, 2] - boxes[:, 0]) * (boxes[:, 3] - boxes[:, 1])
        iw = jnp.minimum(anchor[:, 2][:, None], boxes[None, :, 2]) - jnp.maximum(
            anchor[:, 0][:, None], boxes[None, :, 0])
        ih = jnp.minimum(anchor[:, 3][:, None], boxes[None, :, 3]) - jnp.maximum(
            anchor[:, 1][:, None], boxes[None, :, 1])
        iw = jnp.clip(iw, 0.0)
        ih = jnp.clip(ih, 0.0)
        ua = jnp.clip((aw * ah)[:, None] + barea[None, :] - iw * ih, 1e-8)
        iou = iw * ih / ua
        iou = jnp.where(valid[None, :], iou, -1.0)
        iou_max = iou.max(axis=1)

        j32 = jnp.arange(32, dtype=jnp.int32)
        idxs = jnp.where(iou == iou_max[:, None], j32[None, :], 99)
        iou_arg = idxs.min(axis=1)

        neg = iou_max < 0.4
        pos = iou_max >= 0.7
        packed = (iou_arg.astype(jnp.uint8)
                  | (pos.astype(jnp.uint8) << 7))
        npos = pos.sum()
        nneg = neg.sum()
        keep = jnp.minimum(nneg, 3 * npos)

        # hard-negative mining: sum of top-`keep` scores via 16-way search
        v = jnp.where(neg, -cls1, jnp.float32(-1e2))
        ks = jnp.arange(16, dtype=jnp.float32)

        def body(_, s):
            lo, hi = s
            t = lo + (ks + 1.0) * ((hi - lo) / 17.0)
            c = (v[:, None] >= t[None, :]).sum(axis=0)
            big = c >= keep
            lo2 = jnp.max(jnp.where(big, t, lo))
            hi2 = jnp.min(jnp.where(big, hi, t))
            return lo2, hi2

        lo, _hi = jax.lax.fori_loop(
            0, 5, body, (jnp.float32(-1e2), jnp.float32(64.0)))
        c_lo = ((v >= lo).sum()).astype(jnp.float32)
        s_lo = jnp.where(v >= lo, v, 0.0).sum()
        keep_f = keep.astype(jnp.float32)
        neg_mean = (s_lo - (c_lo - keep_f) * lo) / jnp.maximum(keep_f, 1.0)
        return neg_mean, packed

    def phase_a_core(cls1, ann, anchor):
        return jax.vmap(phase_a, in_axes=(0, 0, None))(cls1, ann, anchor)

    # ---------------- phase B ----------------
    def phase_b(breg, lreg, anc, ann, gt, rowv, npos, has_gt, cls0, neg_mean):
        # breg [K,4], lreg [K,196], anc [K,4], ann [32,200], gt [K] int32
        onehot = (jnp.arange(32, dtype=jnp.int32)[None, :]
                  == gt[:, None]).astype(jnp.float32)
        gb = jnp.einsum('kj,jc->kc', onehot, ann[:, :4],
                        preferred_element_type=jnp.float32)
        gl = jnp.einsum('kj,jc->kc', onehot, ann[:, 4:],
                        preferred_element_type=jnp.float32)

        aw = anc[:, 2] - anc[:, 0]
        ah = anc[:, 3] - anc[:, 1]
        acx = anc[:, 0] + 0.5 * aw
        acy = anc[:, 1] + 0.5 * ah
        gw = gb[:, 2] - gb[:, 0]
        gh = gb[:, 3] - gb[:, 1]
        gcx = gb[:, 0] + 0.5 * gw
        gcy = gb[:, 1] + 0.5 * gh
        tdx = (gcx - acx) / (aw + 1e-14)
        tdy = (gcy - acy) / (ah + 1e-14)
        tdw = jnp.log(jnp.where(gw > 0, gw / aw, 1.0))
        tdh = jnp.log(jnp.where(gh > 0, gh / ah, 1.0))
        bbox_scale = jnp.array([0.1, 0.1, 0.2, 0.2], jnp.float32)
        btgt = jnp.stack([tdx, tdy, tdw, tdh], axis=1) / bbox_scale
        d = jnp.abs(btgt - breg)
        sl1 = jnp.where(d < 1.0, 0.5 * d * d, d - 0.5)
        npos_f = jnp.maximum(npos, 1.0)
        bbox_loss = jnp.where(
            (has_gt > 0) & (npos > 0),
            jnp.where(rowv[:, None] > 0, sl1, 0.0).sum() / (npos_f * 4.0), 0.0)

        even = (jnp.arange(196) % 2) == 0
        ctr = jnp.where(even, acx[:, None], acy[:, None])
        den = jnp.where(even, aw[:, None], ah[:, None]) + 1e-14
        s = jnp.concatenate(
            [jnp.ones(68, jnp.float32), 3.0 * jnp.ones(128, jnp.float32)])
        lposv = (rowv > 0) & (gl.sum(axis=1) > 0)
        nl = lposv.sum()
        ltgt = (gl - ctr) / den / 0.1
        dd = jnp.abs(ltgt * s - lreg * s)
        wing = jnp.where(dd < _OMEGA, _OMEGA * jnp.log1p(dd / _EPS), dd - _WING_C)
        ldm_loss = jnp.where(
            (has_gt > 0) & (nl > 0),
            jnp.where(lposv[:, None], wing, 0.0).sum() /
            (jnp.maximum(nl, 1) * 196), 0.0)
        pos_mean = jnp.where(rowv > 0, -cls0, 0.0).sum() / npos_f
        cls_loss = jnp.where((has_gt > 0) & (npos > 0),
                             pos_mean + neg_mean, 0.0)
        return cls_loss, bbox_loss, ldm_loss

    def phase_b_core(*a):
        return jax.vmap(phase_b)(*a)

    _fns = (jax.pmap(phase_a_core, in_axes=(0, 0, None)),
            jax.pmap(phase_b_core))
    return _fns


def kernel(classifications, bbox_regressions, ldm_regressions, anchors,
           annotations):
    fa, fb = _build()
    spb = _B // _NC
    cls_h = np.asarray(classifications, np.float32)
    cls1 = np.ascontiguousarray(cls_h[:, :, 1]).reshape(_NC, spb, _A)
    ann_h = np.asarray(annotations, np.float32)
    ann = ann_h.reshape(_NC, spb, _N, 200)
    anc_full = np.asarray(anchors, np.float32)[0]

    neg_mean, packed = fa(cls1, ann, anc_full)
    neg_mean = np.asarray(neg_mean, np.float32).reshape(_B)
    packed = np.asarray(packed).reshape(_B, _A)
    pos_m = (packed >> 7) & 1
    arg_m = packed & 0x3F

    breg_h = np.asarray(bbox_regressions, np.float32)
    lreg_h = np.asarray(ldm_regressions, np.float32)

    breg_g = np.zeros((_B, _K, 4), np.float32)
    lreg_g = np.zeros((_B, _K, 196), np.float32)
    anc_g = np.zeros((_B, _K, 4), np.float32)
    anc_g[:, :, 2:] = 1.0  # pad anchors keep logs/denominators finite
    gt_g = np.full((_B, _K), 99, np.int32)  # 99 -> all-zero one-hot row
    rowv = np.zeros((_B, _K), np.float32)
    cls0_g = np.zeros((_B, _K), np.float32)
    nposs = np.zeros((_B,), np.float32)
    hasgt = np.zeros((_B,), np.float32)

    for b in range(_B):
        idx = np.nonzero(pos_m[b])[0]
        n = idx.size
        assert n <= _K, f'npos={n} exceeds K={_K}'
        breg_g[b, :n] = breg_h[b, idx]
        lreg_g[b, :n] = lreg_h[b, idx]
        anc_g[b, :n] = anc_full[idx]
        gt_g[b, :n] = arg_m[b, idx]
        cls0_g[b, :n] = cls_h[b, idx, 0]
        rowv[b, :n] = 1.0
        nposs[b] = n
        hasgt[b] = float((ann_h[b, :, 0] > 0).any())

    sh = lambda x: x.reshape((_NC, spb) + x.shape[1:])
    cls_loss, bbox_loss, ldm_loss = fb(
        sh(breg_g), sh(lreg_g), sh(anc_g), sh(ann_h), sh(gt_g), sh(rowv),
        sh(nposs), sh(hasgt), sh(cls0_g), sh(neg_mean))
    return (np.asarray(cls_loss, np.float32).reshape(_B),
            np.asarray(bbox_loss, np.float32).reshape(_B),
            np.asarray(ldm_loss, np.float32).reshape(_B))


# revision 7
# speedup vs baseline: 1.2429x; 1.1443x over previous
"""RetinaFace-style multi-task loss on 8 Trainium NeuronCores (data-parallel).

Two-phase device pipeline to avoid shipping the 1.25 GB ldm_regressions tensor
through the interconnect when only ~200 positive-anchor rows per sample
contribute to the bbox/landmark losses:

  Phase A (device, pmap over 8 cores x 2 samples): full A x 32 IoU,
    pos/neg classification, hard-negative-mined classification loss
    (top-k sum via 16-way threshold search -- no sort). Exports a single
    uint8 plane per anchor: matched-GT index (low bits) | pos flag (bit 7).
  Host: compact positive indices, slice the needed rows of
    bbox_regressions / ldm_regressions / anchors.
  Phase B (device, pmap): gathers GT boxes/landmarks from annotations via
    one-hot matmul, SmoothL1 bbox loss + wing landmark loss on [256]-row tiles.

All math fp32, mirroring the reference formulas.
"""
import numpy as np

_B, _A, _N = 16, 102400, 32
_NC = 8
_K = 256  # max positives per sample (observed ~200; asserted at runtime)
_OMEGA, _EPS = 3.0, 2.0
_WING_C = _OMEGA - _OMEGA * float(np.log(1.0 + _OMEGA / _EPS))

_fns = None


def _build():
    global _fns
    if _fns is not None:
        return _fns
    import jax
    import jax.numpy as jnp

    # ---------------- phase A ----------------
    def phase_a(cls1, ann, anchor):
        # cls1 [A] (=classifications[:,1]), ann [32,200], anchor [A,4]
        aw = anchor[:, 2] - anchor[:, 0]
        ah = anchor[:, 3] - anchor[:, 1]
        valid = ann[:, 0] > 0
        boxes = ann[:, :4]
        has_gt = jnp.any(valid)

        barea = (boxes[:, 2] - boxes[:, 0]) * (boxes[:, 3] - boxes[:, 1])
        iw = jnp.minimum(anchor[:, 2][:, None], boxes[None, :, 2]) - jnp.maximum(
            anchor[:, 0][:, None], boxes[None, :, 0])
        ih = jnp.minimum(anchor[:, 3][:, None], boxes[None, :, 3]) - jnp.maximum(
            anchor[:, 1][:, None], boxes[None, :, 1])
        iw = jnp.clip(iw, 0.0)
        ih = jnp.clip(ih, 0.0)
        ua = jnp.clip((aw * ah)[:, None] + barea[None, :] - iw * ih, 1e-8)
        iou = iw * ih / ua
        iou = jnp.where(valid[None, :], iou, -1.0)
        iou_max = iou.max(axis=1)

        j32 = jnp.arange(32, dtype=jnp.int32)
        idxs = jnp.where(iou == iou_max[:, None], j32[None, :], 99)
        iou_arg = idxs.min(axis=1)

        neg = iou_max < 0.4
        pos = iou_max >= 0.7
        packed = (iou_arg.astype(jnp.uint8)
                  | (pos.astype(jnp.uint8) << 7))
        npos = pos.sum()
        nneg = neg.sum()
        keep = jnp.minimum(nneg, 3 * npos)

        # hard-negative mining: sum of top-`keep` scores via 16-way search
        v = jnp.where(neg, -cls1, jnp.float32(-1e2))
        ks = jnp.arange(16, dtype=jnp.float32)

        def body(_, s):
            lo, hi = s
            t = lo + (ks + 1.0) * ((hi - lo) / 17.0)
            c = (v[:, None] >= t[None, :]).sum(axis=0)
            big = c >= keep
            lo2 = jnp.max(jnp.where(big, t, lo))
            hi2 = jnp.min(jnp.where(big, hi, t))
            return lo2, hi2

        lo, _hi = jax.lax.fori_loop(
            0, 5, body, (jnp.float32(-1e2), jnp.float32(64.0)))
        c_lo = ((v >= lo).sum()).astype(jnp.float32)
        s_lo = jnp.where(v >= lo, v, 0.0).sum()
        keep_f = keep.astype(jnp.float32)
        neg_mean = (s_lo - (c_lo - keep_f) * lo) / jnp.maximum(keep_f, 1.0)
        return neg_mean, packed

    def phase_a_core(cls1, ann, anchor):
        return jax.vmap(phase_a, in_axes=(0, 0, None))(cls1, ann, anchor)

    # ---------------- phase B ----------------
    def phase_b(breg, lreg, anc, ann, gt, rowv, npos, has_gt, cls0, neg_mean):
        # breg [K,4], lreg [K,196], anc [K,4], ann [32,200], gt [K] int32
        onehot = (jnp.arange(32, dtype=jnp.int32)[None, :]
                  == gt[:, None]).astype(jnp.float32)
        gb = jnp.einsum('kj,jc->kc', onehot, ann[:, :4],
                        preferred_element_type=jnp.float32)
        gl = jnp.einsum('kj,jc->kc', onehot, ann[:, 4:],
                        preferred_element_type=jnp.float32)

        aw = anc[:, 2] - anc[:, 0]
        ah = anc[:, 3] - anc[:, 1]
        acx = anc[:, 0] + 0.5 * aw
        acy = anc[:, 1] + 0.5 * ah
        gw = gb[:, 2] - gb[:, 0]
        gh = gb[:, 3] - gb[:, 1]
        gcx = gb[:, 0] + 0.5 * gw
        gcy = gb[:, 1] + 0.5 * gh
        tdx = (gcx - acx) / (aw + 1e-14)
        tdy = (gcy - acy) / (ah + 1e-14)
        tdw = jnp.log(jnp.where(gw > 0, gw / aw, 1.0))
        tdh = jnp.log(jnp.where(gh > 0, gh / ah, 1.0))
        bbox_scale = jnp.array([0.1, 0.1, 0.2, 0.2], jnp.float32)
        btgt = jnp.stack([tdx, tdy, tdw, tdh], axis=1) / bbox_scale
        d = jnp.abs(btgt - breg)
        sl1 = jnp.where(d < 1.0, 0.5 * d * d, d - 0.5)
        npos_f = jnp.maximum(npos, 1.0)
        bbox_loss = jnp.where(
            (has_gt > 0) & (npos > 0),
            jnp.where(rowv[:, None] > 0, sl1, 0.0).sum() / (npos_f * 4.0), 0.0)

        even = (jnp.arange(196) % 2) == 0
        ctr = jnp.where(even, acx[:, None], acy[:, None])
        den = jnp.where(even, aw[:, None], ah[:, None]) + 1e-14
        s = jnp.concatenate(
            [jnp.ones(68, jnp.float32), 3.0 * jnp.ones(128, jnp.float32)])
        lposv = (rowv > 0) & (gl.sum(axis=1) > 0)
        nl = lposv.sum()
        ltgt = (gl - ctr) / den / 0.1
        dd = jnp.abs(ltgt * s - lreg * s)
        wing = jnp.where(dd < _OMEGA, _OMEGA * jnp.log1p(dd / _EPS), dd - _WING_C)
        ldm_loss = jnp.where(
            (has_gt > 0) & (nl > 0),
            jnp.where(lposv[:, None], wing, 0.0).sum() /
            (jnp.maximum(nl, 1) * 196), 0.0)
        pos_mean = jnp.where(rowv > 0, -cls0, 0.0).sum() / npos_f
        cls_loss = jnp.where((has_gt > 0) & (npos > 0),
                             pos_mean + neg_mean, 0.0)
        return cls_loss, bbox_loss, ldm_loss

    def phase_b_core(*a):
        return jax.vmap(phase_b)(*a)

    _fns = (jax.pmap(phase_a_core, in_axes=(0, 0, None)),
            jax.pmap(phase_b_core))
    return _fns


def kernel(classifications, bbox_regressions, ldm_regressions, anchors,
           annotations):
    fa, fb = _build()
    spb = _B // _NC
    cls_h = np.asarray(classifications, np.float32)
    cls1 = np.ascontiguousarray(cls_h[:, :, 1]).reshape(_NC, spb, _A)
    ann_h = np.asarray(annotations, np.float32)
    ann = ann_h.reshape(_NC, spb, _N, 200)
    anc_full = np.asarray(anchors, np.float32)[0]

    neg_mean, packed = fa(cls1, ann, anc_full)
    neg_mean = np.asarray(neg_mean, np.float32).reshape(_B)
    packed = np.asarray(packed).reshape(_B, _A)
    pos_m = (packed >> 7) & 1
    arg_m = packed & 0x3F

    breg_h = np.asarray(bbox_regressions, np.float32)
    lreg_h = np.asarray(ldm_regressions, np.float32)

    breg_g = np.zeros((_B, _K, 4), np.float32)
    lreg_g = np.zeros((_B, _K, 196), np.float32)
    anc_g = np.zeros((_B, _K, 4), np.float32)
    anc_g[:, :, 2:] = 1.0  # pad anchors keep logs/denominators finite
    gt_g = np.full((_B, _K), 99, np.int32)  # 99 -> all-zero one-hot row
    rowv = np.zeros((_B, _K), np.float32)
    cls0_g = np.zeros((_B, _K), np.float32)
    nposs = np.zeros((_B,), np.float32)
    hasgt = np.zeros((_B,), np.float32)

    for b in range(_B):
        idx = np.nonzero(pos_m[b])[0]
        n = idx.size
        assert n <= _K, f'npos={n} exceeds K={_K}'
        breg_g[b, :n] = breg_h[b, idx]
        lreg_g[b, :n] = lreg_h[b, idx]
        anc_g[b, :n] = anc_full[idx]
        gt_g[b, :n] = arg_m[b, idx]
        cls0_g[b, :n] = cls_h[b, idx, 0]
        rowv[b, :n] = 1.0
        nposs[b] = n
        hasgt[b] = float((ann_h[b, :, 0] > 0).any())

    sh = lambda x: x.reshape((_NC, spb) + x.shape[1:])
    cls_loss, bbox_loss, ldm_loss = fb(
        sh(breg_g), sh(lreg_g), sh(anc_g), sh(ann_h), sh(gt_g), sh(rowv),
        sh(nposs), sh(hasgt), sh(cls0_g), sh(neg_mean))
    return (np.asarray(cls_loss, np.float32).reshape(_B),
            np.asarray(bbox_loss, np.float32).reshape(_B),
            np.asarray(ldm_loss, np.float32).reshape(_B))
